# revision 67
# baseline (speedup 1.0000x reference)
"""Multi-head attention on 8 Trainium2 NeuronCores.

Sharding: 2-way data parallel over batch x 4-way tensor parallel over heads
(4 heads/core). Per-core device kernel, for its (batch, head-group):

  phase A : q^T = (x_q @ Wq + bq)^T, k^T likewise (feature-major, fp16),
            pipelined by token half so phase B starts after only the
            q-half0/k-half0 projections; the half-1 projections and the
            v projection ride inside the first two units' score streams.
  phase B : per (q-half, head) unit: s^T = k q^T (transposed-score layout),
            e^T = exp(s^T/8) (bf16), masked in place (DVE/Pool split), then
            PV in the swapped orientation: acc[q, 0:65] += em^T.T @ [v|1]
            per 128-token q-chunk -- the moving operand is the 65-wide
            [v|1], so PV costs 65 PE columns per (strip, chunk) instead of
            1024 per strip, and the ones column gives softmax row-sums for
            free, aligned per-partition.  Normalize via DVE reciprocal +
            per-partition tensor_scalar mul, then PE-transpose head-pair
            x tiles back to feature-major for phase C.  Unit u+1's scores
            are emitted before unit u's PV so the in-order PE queue never
            starves the Activation engine (the exp stream is the global
            bottleneck).
  phase C : partial_out = x^T.T @ Wo_rows (row-parallel Wo), interleaved.

DMAs are batched (multi-strip access patterns) to stay off the SP
sequencer's 565ns-per-DMA serial cost. Host: shards/transposes inputs
(fp16), sums the 4 group partials per batch, adds bo.
"""
import numpy as np
import ml_dtypes

import concourse.bass as bass
import concourse.bacc as bacc
import concourse.tile as tile
from concourse import mybir
from concourse.bass_utils import run_bass_kernel_spmd

B, S, D, H = 2, 2048, 1024, 16
DK = 64                    # head dim
GROUPS = 4                 # head-group tensor parallel factor
HL = H // GROUPS           # heads per core
DH = HL * DK               # 256 local features
NCORES = 8
NK = D // 128              # 8 contraction chunks
NJ = S // 128              # 16 token strips
SC = 512                   # matmul moving-operand chunk
HS = S // 2                # 1024: q-half size in phase B
NQC = HS // 128            # 8 q-chunks per half
F32 = mybir.dt.float32
F16 = mybir.dt.float16
BF16 = mybir.dt.bfloat16
AF = mybir.ActivationFunctionType

# mask-mul strips offloaded from DVE to the (otherwise idle) GPSIMD engine
POOL_JS = (2, 5, 8, 11, 13)

_CACHE = {}


def _build():
    nc = bacc.Bacc("TRN2")
    xqT = nc.dram_tensor("xqT", (D, S), F16, kind="ExternalInput")
    xkT = nc.dram_tensor("xkT", (D, S), F16, kind="ExternalInput")
    xvT = nc.dram_tensor("xvT", (D, S), F16, kind="ExternalInput")
    mT = nc.dram_tensor("mT", (S, S), BF16, kind="ExternalInput")
    wq = nc.dram_tensor("wq", (D, DH), F16, kind="ExternalInput")
    wk = nc.dram_tensor("wk", (D, DH), F16, kind="ExternalInput")
    wv = nc.dram_tensor("wv", (D + 1, DH), F16, kind="ExternalInput")
    wo = nc.dram_tensor("wo", (DH, D), F16, kind="ExternalInput")
    bqk = nc.dram_tensor("bqk", (128, 4), F32, kind="ExternalInput")
    ident = nc.dram_tensor("ident", (128, 128), F16, kind="ExternalInput")
    out = nc.dram_tensor("out", (S, D), BF16, kind="ExternalOutput")

    F8 = mybir.dt.float8e4
    DR = mybir.MatmulPerfMode.DoubleRow
    with tile.TileContext(nc) as tc:
        with tc.tile_pool(name="sp", bufs=1) as sp:
            qT = sp.tile([128, 2, S], F16)
            kT = sp.tile([128, 2, S], F16)
            vta = sp.tile([128, NJ, HL, DK + 1], BF16, name="vta")
            woS = sp.tile([128, 2, D], F16)
            xfin = sp.tile([128, 2, S], F16)
            identS = sp.tile([128, 128], F16)
            ones = sp.tile([1, SC], F16)
            zlhs = sp.tile([1, 128], F16)
            biasT = sp.tile([128, 4], F32)
            nc.vector.memset(ones, 1.0)
            nc.vector.memset(zlhs, 0.0)
            nc.vector.memset(vta[:, :, :, DK:DK + 1], 1.0)

            pa = tc.alloc_tile_pool(name="pa", bufs=1)
            wvS = pa.tile([128, NK + 1, DH], F16, name="wvS", bufs=1)
            wqS = pa.tile([128, NK, DH], F16, name="wqS", bufs=1)
            wkS = pa.tile([128, NK, DH], F16, name="wkS", bufs=1)

            def load_w(dst, src):
                nc.sync.dma_start(
                    out=dst[0:128, 0:NK, :],
                    in_=src[0:D, :].rearrange("(kc p) d -> p kc d", p=128))

            def xhalf_part(xs, xT, half, c):
                k0 = c * (NK // 2)
                off = half * HS
                nc.sync.dma_start(
                    out=xs[:, k0:k0 + NK // 2, :],
                    in_=xT[k0 * 128:(k0 + NK // 2) * 128,
                           off:off + HS].rearrange(
                        "(kc p) s -> p kc s", p=128))

            def load_xhalf(xT, half):
                """One token-half of an input, batched into two DMAs so the
                projection can start on the first four chunks."""
                xs = pa.tile([128, NK, HS], F16, name="xs", tag="xs", bufs=2)
                for c in range(2):
                    xhalf_part(xs, xT, half, c)
                return xs

            mgs = {}

            def load_mask_grp(g, half):
                mg = pa.tile([128, 4, HS], BF16, name="mg", tag="mg",
                             bufs=4)
                j0 = 4 * g
                off = half * HS
                nc.sync.dma_start(
                    out=mg[:, :, :],
                    in_=mT[j0 * 128:(j0 + 4) * 128,
                           off:off + HS].rearrange("(j p) s -> p j s",
                                                   p=128))
                mgs[(half, g)] = mg

            # --- startup DMA, need-ordered ---
            nc.sync.dma_start(out=biasT, in_=bqk[:, :])
            load_w(wqS, wq)
            xq0 = load_xhalf(xqT, 0)
            load_w(wkS, wk)
            xk0 = load_xhalf(xkT, 0)
            load_mask_grp(0, 0)

            psA = tc.alloc_tile_pool(name="psA", bufs=1, space="PSUM")

            def proj_half(name, xs, wS, dst, half, engs, ms=(0, 1),
                          ns=(0, 1)):
                """Project one token-half of q or k. Half-bank psum tiles
                (projps bufs=2) let chunk n+1 project while n evicts."""
                off = half * HS
                bc0 = 0 if name == "q" else 2
                for m in ms:
                    for n in ns:
                        ps = psA.tile([128, SC], F32,
                                      name=f"ps{name}{half}{m}{n}",
                                      tag="projps", bufs=2)
                        for kc in range(NK):
                            nc.tensor.matmul(
                                out=ps[:, :],
                                lhsT=wS[0:128, kc, m * 128:(m + 1) * 128],
                                rhs=xs[0:128, kc, n * SC:(n + 1) * SC],
                                start=(kc == 0), stop=(kc == NK - 1))
                        dslice = dst[:, m, off + n * SC:off + (n + 1) * SC]
                        if engs[n % 2] == "act":
                            nc.scalar.activation(
                                dslice, ps, AF.Identity,
                                bias=biasT[:, bc0 + m:bc0 + m + 1])
                        else:
                            with nc.allow_low_precision(
                                    reason="bias add into fp16 eviction"):
                                nc.vector.tensor_scalar_add(
                                    dslice, ps,
                                    biasT[:, bc0 + m:bc0 + m + 1])

            proj_half("q", xq0, wqS, qT, 0, ("act", "dve"))
            proj_half("k", xk0, wkS, kT, 0, ("act", "dve"), ms=(0,))

            # rest of the input stream (SP queue order = need order),
            # interleaved at half-tensor granularity against the mask
            # strips the exp stream consumes at ~1 strip/us
            xk1 = pa.tile([128, NK, HS], F16, name="xk1", tag="xs", bufs=2)
            xhalf_part(xk1, xkT, 1, 0)
            load_mask_grp(1, 0)
            xhalf_part(xk1, xkT, 1, 1)
            load_mask_grp(2, 0)
            load_mask_grp(3, 0)
            nc.sync.dma_start(
                out=wvS[0:128, 0:NK, :],
                in_=wv[0:D, :].rearrange("(kc p) d -> p kc d", p=128))
            nc.sync.dma_start(out=wvS[0:1, NK, :], in_=wv[D:D + 1, :])
            xv0 = pa.tile([128, NK, HS], F16, name="xv0", tag="xs", bufs=2)
            xhalf_part(xv0, xvT, 0, 0)
            xhalf_part(xv0, xvT, 0, 1)
            xv1 = pa.tile([128, NK, HS], F16, name="xv1", tag="xs", bufs=2)
            xq1 = pa.tile([128, NK, HS], F16, name="xq1", tag="xs", bufs=2)
            xhalf_part(xv1, xvT, 1, 0)
            xhalf_part(xq1, xqT, 1, 0)
            xhalf_part(xv1, xvT, 1, 1)
            xhalf_part(xq1, xqT, 1, 1)
            nc.sync.dma_start(
                out=woS[:, :, :],
                in_=wo[:, :].rearrange("(s p) d -> p s d", p=128))
            nc.sync.dma_start(out=identS, in_=ident[:, :])
            for g in range(4):
                load_mask_grp(g, 1)
            xvs = (xv0, xv1)

            # ---------------- phase B ----------------
            psB = tc.alloc_tile_pool(name="psB", bufs=1, space="PSUM")
            pb = tc.alloc_tile_pool(name="pb", bufs=1)

            units = [(half, h) for half in range(2) for h in range(HL)]
            eTs_of = {}

            def emit_smem(h, half, j, po_, hs_):
                # scores (fp8 DoubleRow) -> exp -> in-place mask-mul for one
                # (unit, j) strip.  Quarter-bank output chunks: the second
                # chunk of each PSUM bank accumulates with start=False onto
                # the bank's pending-zero region.
                off = half * HS
                sT = psB.tile([128, HS], F32, name="sT", tag="big", bufs=2)
                for c2 in range(2):
                    nc.tensor.matmul(
                        out=sT[:, c2 * SC:(c2 + 1) * SC],
                        lhsT=kT[po_:po_ + DK, hs_,
                                j * 128:(j + 1) * 128],
                        rhs=qT[po_:po_ + DK, hs_,
                               off + c2 * SC:off + (c2 + 1) * SC],
                        start=True, stop=True)
                eT = pb.tile([128, HS], BF16, name="eT", tag="eT", bufs=36)
                nc.scalar.activation(eT, sT, AF.Exp, scale=0.125)
                eng = nc.gpsimd if j in POOL_JS else nc.vector
                eng.tensor_mul(eT, eT, mgs[(half, j // 4)][:, j % 4, :])
                return eT

            def emit_vstrip(m):
                # v projection for one 128-token strip (token-major + ones);
                # psum rides the otherwise-idle projps bank
                grp, m8 = m // 8, m % 8
                pv = psA.tile([128, DH], F32, name="pv", tag="projps",
                              bufs=2)
                for kc in range(NK + 1):
                    if kc < NK:
                        lhsT = xvs[grp][:, kc, m8 * 128:(m8 + 1) * 128]
                    else:
                        lhsT = ones[0:1, 0:128]
                    nc.tensor.matmul(
                        out=pv[:, :],
                        lhsT=lhsT,
                        rhs=wvS[0:(128 if kc < NK else 1), kc, :],
                        start=(kc == 0), stop=(kc == NK))
                nc.vector.tensor_copy(
                    out=vta[:, m, :, 0:DK],
                    in_=pv[:, :].rearrange("p (h d) -> p h d", h=HL))

            def emit_scores_range(i, j0, j1, hooks=None):
                half, h = units[i]
                eTs = eTs_of.setdefault(i, [])
                for j in range(j0, j1):
                    eTs.append(emit_smem(h, half, j, (h % 2) * DK, h // 2))
                    if hooks and j in hooks:
                        hooks[j]()

            def emit_scores_unit(i, hooks=None):
                emit_scores_range(i, 0, NJ, hooks)

            def make_accs():
                # two acc tiles (1 PSUM bank each) hold the 8 q-chunk
                # accumulators; a zero-matmul clears each bank so the
                # per-chunk accumulation can run with start=False
                accs = []
                for a in range(2):
                    acc = psB.tile([128, 4, 128], F32, name="acc",
                                   tag="small", bufs=2)
                    nc.tensor.matmul(
                        out=acc[:, :, :], lhsT=zlhs[0:1, 0:128],
                        rhs=ones[0:1, 0:SC], start=True, stop=True)
                    accs.append(acc)
                return accs

            def pv_mm(i, h, accs, j, qc):
                nc.tensor.matmul(
                    out=accs[qc // 4][:, qc % 4, 0:DK + 1],
                    lhsT=eTs_of[i][j][:, qc * 128:(qc + 1) * 128],
                    rhs=vta[:, j, h, :],
                    start=False, stop=(j == NJ - 1),
                    skip_group_check=True)

            def emit_pv_strips(i, accs, jr, with_v=False):
                half, h = units[i]
                for j in jr:
                    if with_v:
                        # v-strip j feeds pv strip j directly: an xv DMA
                        # stall here never blocks a score stream
                        emit_vstrip(j)
                    for qc in range(NQC):
                        pv_mm(i, h, accs, j, qc)

            xps_of = {}

            def emit_tp(half, h, xps, qc, tail=False):
                # transpose one head-pair x tile chunk to feature-major xfin
                pair, off = h // 2, half * HS
                tp = psB.tile([128, 8, 128], F16, name="tp",
                              tag="small", bufs=2)
                nc.tensor.transpose(tp[:, 0, :], xps[qc][:, :, :], identS)
                nc.vector.tensor_copy(
                    out=xfin[:, pair, off + qc * 128:off + (qc + 1) * 128],
                    in_=tp[:, 0, :])

            def emit_norm_unit(i, accs, fuse_tail=False):
                half, h = units[i]
                if h % 2 == 0:
                    xps = [pb.tile([128, 2, DK], F16, name="xp",
                                   tag="xp", bufs=16) for _ in range(NQC)]
                    xps_of[h // 2] = xps
                else:
                    xps = xps_of[h // 2]
                for qc in range(NQC):
                    acc = accs[qc // 4]
                    rec = pb.tile([128, 1], F32, name="rec", tag="rec",
                                  bufs=8)
                    with nc.allow_low_precision(
                            reason="softmax denom reciprocal"):
                        nc.vector.reciprocal(rec, acc[:, qc % 4, DK:DK + 1])
                        nc.vector.tensor_scalar_mul(
                            xps[qc][:, h % 2, :], acc[:, qc % 4, 0:DK], rec)
                    if fuse_tail:
                        # final unit: stream norm -> transpose -> out-proj
                        # per q-chunk to shorten the kernel tail
                        emit_tp(half, h, xps, qc, tail=True)
                        out_proj(NQC + qc, ("act", "dve"), tail=True,
                                 alt=qc % 2)
                if h % 2 == 1 and not fuse_tail:
                    for qc in range(NQC):
                        emit_tp(half, h, xps, qc)

            def out_proj(m, engs, tail=False, alt=0):
                # phase C: one 128-token output strip.  During phase B the
                # psum rides half-bank tiles on the "small" ring (keeps the
                # sT double-buffer undisturbed); at the tail the score ring
                # is idle, so a full-width tile + one eviction is cheaper.
                ost = pb.tile([128, D], BF16, name="ost", tag="ost", bufs=2)
                if tail:
                    po = psB.tile([128, D], F32, name="pot",
                                  tag="big", bufs=2)
                    for kc in range(2):
                        for n2 in range(2):
                            nc.tensor.matmul(
                                out=po[:, n2 * SC:(n2 + 1) * SC],
                                lhsT=xfin[:, kc, m * 128:(m + 1) * 128],
                                rhs=woS[:, kc, n2 * SC:(n2 + 1) * SC],
                                start=(kc == 0), stop=(kc == 1))
                    if alt:
                        nc.scalar.activation(ost, po, AF.Copy)
                    else:
                        nc.vector.tensor_copy(out=ost, in_=po)
                else:
                    for n2 in range(2):
                        po = psB.tile([128, SC], F32, name="po2",
                                      tag="small", bufs=2)
                        for kc in range(2):
                            nc.tensor.matmul(
                                out=po,
                                lhsT=xfin[:, kc, m * 128:(m + 1) * 128],
                                rhs=woS[:, kc, n2 * SC:(n2 + 1) * SC],
                                start=(kc == 0), stop=(kc == 1))
                        oslice = ost[:, n2 * SC:(n2 + 1) * SC]
                        if engs[n2] == "act":
                            nc.scalar.activation(oslice, po, AF.Copy)
                        else:
                            nc.vector.tensor_copy(out=oslice, in_=po)
                nc.sync.dma_start(out=out[m * 128:(m + 1) * 128, :],
                                  in_=ost)

            # --- software-pipelined phase B schedule ---
            # Unit u's scores are emitted between the two halves of unit
            # u-1's PV so the eT ring slots for u's first exps free up
            # before u's score stream occupies the PE queue.
            # scores run two units ahead of PV so neither the v-projection
            # block nor a PV stream ever delays the exp-feeding score
            # stream in the in-order PE queue
            emit_scores_unit(0, hooks={
                3: lambda: proj_half("k", xk0, wkS, kT, 0,
                                     ("dve", "dve"), ms=(1,)),
                5: lambda: proj_half("k", xk1, wkS, kT, 1,
                                     ("dve", "dve"), ms=(0,)),
                12: lambda: proj_half("k", xk1, wkS, kT, 1,
                                      ("dve", "dve"), ms=(1,))})
            emit_scores_unit(1)
            emit_scores_unit(2)
            accs0 = make_accs()
            emit_pv_strips(0, accs0, range(NJ), with_v=True)
            emit_norm_unit(0, accs0)
            pend = [1, 2]
            op_m = 0
            for i in range(3, len(units)):
                pi = pend.pop(0)
                paccs = make_accs()
                emit_pv_strips(pi, paccs, range(0, NJ // 2))
                emit_scores_unit(i)
                emit_pv_strips(pi, paccs, range(NJ // 2, NJ))
                emit_norm_unit(pi, paccs)
                if pi == 1:
                    # half-1 q projection, needed from unit 4 on
                    proj_half("q", xq1, wqS, qT, 1, ("dve", "dve"))
                if pi >= 3:
                    # out-proj strips interleave once half 0 is complete
                    # (unit 3 closes half 0's xfin)
                    for _ in range(2):
                        out_proj(op_m, ("dve", "dve"))
                        op_m += 1
                pend.append(i)
            pi = pend.pop(0)
            paccs = make_accs()
            emit_pv_strips(pi, paccs, range(NJ))
            emit_norm_unit(pi, paccs)
            for _ in range(2):
                out_proj(op_m, ("dve", "dve"))
                op_m += 1
            # final unit: qc-outer PV fused with norm -> transpose ->
            # out-proj per q-chunk, so the tail pipeline starts with the
            # first q-chunk instead of after the whole PV block
            li = pend.pop(0)
            laccs = make_accs()
            lhalf, lh = units[li]
            lxps = xps_of[lh // 2]
            # alternate acc tiles (qc 0,4,1,5,...) so each chunk's PV
            # matmuls overlap the other tile's normalize chain instead of
            # serializing on whole-tile read/write ordering
            for qc in [x for p in zip(range(4), range(4, 8)) for x in p]:
                for j in range(NJ):
                    pv_mm(li, lh, laccs, j, qc)
                acc = laccs[qc // 4]
                rec = pb.tile([128, 1], F32, name="rec", tag="rec", bufs=8)
                with nc.allow_low_precision(
                        reason="softmax denom reciprocal"):
                    nc.vector.reciprocal(rec, acc[:, qc % 4, DK:DK + 1])
                    nc.vector.tensor_scalar_mul(
                        lxps[qc][:, lh % 2, :], acc[:, qc % 4, 0:DK], rec)
                emit_tp(lhalf, lh, lxps, qc, tail=True)
                out_proj(NQC + qc, ("act", "dve"), tail=True, alt=qc % 2)
            pb.release()
            psB.release()
            psA.release()
            pa.release()
    nc.finalize()
    return nc


def _get_nc():
    if "nc" not in _CACHE:
        _CACHE["nc"] = _build()
    return _CACHE["nc"]


def _prep_in_maps(query, key_, value, mask, Wq, bq, Wk, bk, Wv, bv, Wo, bo):
    query = np.asarray(query, np.float32)
    key_ = np.asarray(key_, np.float32)
    value = np.asarray(value, np.float32)
    mask = np.asarray(mask)
    Wq, bq = np.asarray(Wq, np.float32), np.asarray(bq, np.float32)
    Wk, bk = np.asarray(Wk, np.float32), np.asarray(bk, np.float32)
    Wv, bv = np.asarray(Wv, np.float32), np.asarray(bv, np.float32)
    Wo = np.asarray(Wo, np.float32)

    xT = {}
    for b in range(B):
        xT[("q", b)] = np.ascontiguousarray(query[b].T).astype(np.float16)
        xT[("k", b)] = np.ascontiguousarray(key_[b].T).astype(np.float16)
        xT[("v", b)] = np.ascontiguousarray(value[b].T).astype(np.float16)
        xT[("m", b)] = np.ascontiguousarray(mask[b].T).astype(
            ml_dtypes.bfloat16)
    identity = np.eye(128, dtype=np.float16)
    wg = {}
    for g in range(GROUPS):
        c0, c1 = g * DH, (g + 1) * DH
        wg[("q", g)] = np.ascontiguousarray(Wq[:, c0:c1]).astype(np.float16)
        wg[("k", g)] = np.ascontiguousarray(Wk[:, c0:c1]).astype(np.float16)
        wg[("v", g)] = np.concatenate(
            [Wv[:, c0:c1], bv[None, c0:c1]], axis=0).astype(np.float16)
        wg[("o", g)] = np.ascontiguousarray(Wo[c0:c1, :]).astype(np.float16)
        wg[("bqk", g)] = np.stack(
            [bq[c0:c0 + 128], bq[c0 + 128:c1],
             bk[c0:c0 + 128], bk[c0 + 128:c1]], axis=1).astype(np.float32)

    in_maps = []
    for c in range(NCORES):
        b, g = c // GROUPS, c % GROUPS
        in_maps.append({
            "xqT": xT[("q", b)], "xkT": xT[("k", b)], "xvT": xT[("v", b)],
            "mT": xT[("m", b)],
            "wq": wg[("q", g)], "wk": wg[("k", g)], "wv": wg[("v", g)],
            "wo": wg[("o", g)], "bqk": wg[("bqk", g)],
            "ident": identity,
        })
    return in_maps


def _gather(results, bo):
    bo = np.asarray(bo, np.float32)
    outs = []
    for b in range(B):
        acc = results[b * GROUPS]["out"].astype(np.float32).copy()
        for g in range(1, GROUPS):
            acc += results[b * GROUPS + g]["out"]
        outs.append(acc + bo[None, :])
    return np.stack(outs, axis=0)


def run(trace=False, **inputs):
    in_maps = _prep_in_maps(**inputs)
    nc = _get_nc()
    res = run_bass_kernel_spmd(nc, in_maps, core_ids=list(range(NCORES)),
                               trace=trace)
    out = _gather(res.results, inputs["bo"])
    return out, res


def kernel(**inputs) -> np.ndarray:
    out, _ = run(trace=False, **inputs)
    return out


# revision 68
# speedup vs baseline: 1.0002x; 1.0002x over previous
"""Multi-head attention on 8 Trainium2 NeuronCores.

Sharding: 2-way data parallel over batch x 4-way tensor parallel over heads
(4 heads/core). Per-core device kernel, for its (batch, head-group):

  phase A : q^T = (x_q @ Wq + bq)^T, k^T likewise (feature-major, fp16),
            pipelined by token half so phase B starts after only the
            q-half0/k-half0 projections; the half-1 projections and the
            v projection ride inside the first two units' score streams.
  phase B : per (q-half, head) unit: s^T = k q^T (transposed-score layout),
            e^T = exp(s^T/8) (bf16), masked in place (DVE/Pool split), then
            PV in the swapped orientation: acc[q, 0:65] += em^T.T @ [v|1]
            per 128-token q-chunk -- the moving operand is the 65-wide
            [v|1], so PV costs 65 PE columns per (strip, chunk) instead of
            1024 per strip, and the ones column gives softmax row-sums for
            free, aligned per-partition.  Normalize via DVE reciprocal +
            per-partition tensor_scalar mul, then PE-transpose head-pair
            x tiles back to feature-major for phase C.  Unit u+1's scores
            are emitted before unit u's PV so the in-order PE queue never
            starves the Activation engine (the exp stream is the global
            bottleneck).
  phase C : partial_out = x^T.T @ Wo_rows (row-parallel Wo), interleaved.

DMAs are batched (multi-strip access patterns) to stay off the SP
sequencer's 565ns-per-DMA serial cost. Host: shards/transposes inputs
(fp16), sums the 4 group partials per batch, adds bo.
"""
import numpy as np
import ml_dtypes

import concourse.bass as bass
import concourse.bacc as bacc
import concourse.tile as tile
from concourse import mybir
from concourse.bass_utils import run_bass_kernel_spmd

B, S, D, H = 2, 2048, 1024, 16
DK = 64                    # head dim
GROUPS = 4                 # head-group tensor parallel factor
HL = H // GROUPS           # heads per core
DH = HL * DK               # 256 local features
NCORES = 8
NK = D // 128              # 8 contraction chunks
NJ = S // 128              # 16 token strips
SC = 512                   # matmul moving-operand chunk
HS = S // 2                # 1024: q-half size in phase B
NQC = HS // 128            # 8 q-chunks per half
F32 = mybir.dt.float32
F16 = mybir.dt.float16
BF16 = mybir.dt.bfloat16
AF = mybir.ActivationFunctionType

# mask-mul strips offloaded from DVE to the (otherwise idle) GPSIMD engine
POOL_JS = (2, 5, 8, 11, 14)

_CACHE = {}


def _build():
    nc = bacc.Bacc("TRN2")
    xqT = nc.dram_tensor("xqT", (D, S), F16, kind="ExternalInput")
    xkT = nc.dram_tensor("xkT", (D, S), F16, kind="ExternalInput")
    xvT = nc.dram_tensor("xvT", (D, S), F16, kind="ExternalInput")
    mT = nc.dram_tensor("mT", (S, S), BF16, kind="ExternalInput")
    wq = nc.dram_tensor("wq", (D, DH), F16, kind="ExternalInput")
    wk = nc.dram_tensor("wk", (D, DH), F16, kind="ExternalInput")
    wv = nc.dram_tensor("wv", (D + 1, DH), F16, kind="ExternalInput")
    wo = nc.dram_tensor("wo", (DH, D), F16, kind="ExternalInput")
    bqk = nc.dram_tensor("bqk", (128, 4), F32, kind="ExternalInput")
    ident = nc.dram_tensor("ident", (128, 128), F16, kind="ExternalInput")
    out = nc.dram_tensor("out", (S, D), BF16, kind="ExternalOutput")

    F8 = mybir.dt.float8e4
    DR = mybir.MatmulPerfMode.DoubleRow
    with tile.TileContext(nc) as tc:
        with tc.tile_pool(name="sp", bufs=1) as sp:
            qT = sp.tile([128, 2, S], F16)
            kT = sp.tile([128, 2, S], F16)
            vta = sp.tile([128, NJ, HL, DK + 1], BF16, name="vta")
            woS = sp.tile([128, 2, D], F16)
            xfin = sp.tile([128, 2, S], F16)
            identS = sp.tile([128, 128], F16)
            ones = sp.tile([1, SC], F16)
            zlhs = sp.tile([1, 128], F16)
            biasT = sp.tile([128, 4], F32)
            nc.vector.memset(ones, 1.0)
            nc.vector.memset(zlhs, 0.0)
            nc.vector.memset(vta[:, :, :, DK:DK + 1], 1.0)

            pa = tc.alloc_tile_pool(name="pa", bufs=1)
            wvS = pa.tile([128, NK + 1, DH], F16, name="wvS", bufs=1)
            wqS = pa.tile([128, NK, DH], F16, name="wqS", bufs=1)
            wkS = pa.tile([128, NK, DH], F16, name="wkS", bufs=1)

            def load_w(dst, src):
                nc.sync.dma_start(
                    out=dst[0:128, 0:NK, :],
                    in_=src[0:D, :].rearrange("(kc p) d -> p kc d", p=128))

            def xhalf_part(xs, xT, half, c):
                k0 = c * (NK // 2)
                off = half * HS
                nc.sync.dma_start(
                    out=xs[:, k0:k0 + NK // 2, :],
                    in_=xT[k0 * 128:(k0 + NK // 2) * 128,
                           off:off + HS].rearrange(
                        "(kc p) s -> p kc s", p=128))

            def load_xhalf(xT, half):
                """One token-half of an input, batched into two DMAs so the
                projection can start on the first four chunks."""
                xs = pa.tile([128, NK, HS], F16, name="xs", tag="xs", bufs=2)
                for c in range(2):
                    xhalf_part(xs, xT, half, c)
                return xs

            mgs = {}

            def load_mask_grp(g, half):
                mg = pa.tile([128, 4, HS], BF16, name="mg", tag="mg",
                             bufs=4)
                j0 = 4 * g
                off = half * HS
                nc.sync.dma_start(
                    out=mg[:, :, :],
                    in_=mT[j0 * 128:(j0 + 4) * 128,
                           off:off + HS].rearrange("(j p) s -> p j s",
                                                   p=128))
                mgs[(half, g)] = mg

            # --- startup DMA, need-ordered ---
            nc.sync.dma_start(out=biasT, in_=bqk[:, :])
            load_w(wqS, wq)
            xq0 = load_xhalf(xqT, 0)
            load_w(wkS, wk)
            xk0 = load_xhalf(xkT, 0)
            load_mask_grp(0, 0)

            psA = tc.alloc_tile_pool(name="psA", bufs=1, space="PSUM")

            def proj_half(name, xs, wS, dst, half, engs, ms=(0, 1),
                          ns=(0, 1)):
                """Project one token-half of q or k. Half-bank psum tiles
                (projps bufs=2) let chunk n+1 project while n evicts."""
                off = half * HS
                bc0 = 0 if name == "q" else 2
                for m in ms:
                    for n in ns:
                        ps = psA.tile([128, SC], F32,
                                      name=f"ps{name}{half}{m}{n}",
                                      tag="projps", bufs=2)
                        for kc in range(NK):
                            nc.tensor.matmul(
                                out=ps[:, :],
                                lhsT=wS[0:128, kc, m * 128:(m + 1) * 128],
                                rhs=xs[0:128, kc, n * SC:(n + 1) * SC],
                                start=(kc == 0), stop=(kc == NK - 1))
                        dslice = dst[:, m, off + n * SC:off + (n + 1) * SC]
                        if engs[n % 2] == "act":
                            nc.scalar.activation(
                                dslice, ps, AF.Identity,
                                bias=biasT[:, bc0 + m:bc0 + m + 1])
                        else:
                            with nc.allow_low_precision(
                                    reason="bias add into fp16 eviction"):
                                nc.vector.tensor_scalar_add(
                                    dslice, ps,
                                    biasT[:, bc0 + m:bc0 + m + 1])

            proj_half("q", xq0, wqS, qT, 0, ("act", "dve"))
            proj_half("k", xk0, wkS, kT, 0, ("act", "dve"), ms=(0,))

            # rest of the input stream (SP queue order = need order),
            # interleaved at half-tensor granularity against the mask
            # strips the exp stream consumes at ~1 strip/us
            xk1 = pa.tile([128, NK, HS], F16, name="xk1", tag="xs", bufs=2)
            xhalf_part(xk1, xkT, 1, 0)
            load_mask_grp(1, 0)
            xhalf_part(xk1, xkT, 1, 1)
            load_mask_grp(2, 0)
            load_mask_grp(3, 0)
            nc.sync.dma_start(
                out=wvS[0:128, 0:NK, :],
                in_=wv[0:D, :].rearrange("(kc p) d -> p kc d", p=128))
            nc.sync.dma_start(out=wvS[0:1, NK, :], in_=wv[D:D + 1, :])
            xv0 = pa.tile([128, NK, HS], F16, name="xv0", tag="xs", bufs=2)
            xhalf_part(xv0, xvT, 0, 0)
            xhalf_part(xv0, xvT, 0, 1)
            xv1 = pa.tile([128, NK, HS], F16, name="xv1", tag="xs", bufs=2)
            xq1 = pa.tile([128, NK, HS], F16, name="xq1", tag="xs", bufs=2)
            xhalf_part(xv1, xvT, 1, 0)
            xhalf_part(xq1, xqT, 1, 0)
            xhalf_part(xv1, xvT, 1, 1)
            xhalf_part(xq1, xqT, 1, 1)
            nc.sync.dma_start(
                out=woS[:, :, :],
                in_=wo[:, :].rearrange("(s p) d -> p s d", p=128))
            nc.sync.dma_start(out=identS, in_=ident[:, :])
            for g in range(4):
                load_mask_grp(g, 1)
            xvs = (xv0, xv1)

            # ---------------- phase B ----------------
            psB = tc.alloc_tile_pool(name="psB", bufs=1, space="PSUM")
            pb = tc.alloc_tile_pool(name="pb", bufs=1)

            units = [(half, h) for half in range(2) for h in range(HL)]
            eTs_of = {}

            def emit_smem(h, half, j, po_, hs_):
                # scores (fp8 DoubleRow) -> exp -> in-place mask-mul for one
                # (unit, j) strip.  Quarter-bank output chunks: the second
                # chunk of each PSUM bank accumulates with start=False onto
                # the bank's pending-zero region.
                off = half * HS
                sT = psB.tile([128, HS], F32, name="sT", tag="big", bufs=2)
                for c2 in range(2):
                    nc.tensor.matmul(
                        out=sT[:, c2 * SC:(c2 + 1) * SC],
                        lhsT=kT[po_:po_ + DK, hs_,
                                j * 128:(j + 1) * 128],
                        rhs=qT[po_:po_ + DK, hs_,
                               off + c2 * SC:off + (c2 + 1) * SC],
                        start=True, stop=True)
                eT = pb.tile([128, HS], BF16, name="eT", tag="eT", bufs=36)
                nc.scalar.activation(eT, sT, AF.Exp, scale=0.125)
                eng = nc.gpsimd if j in POOL_JS else nc.vector
                eng.tensor_mul(eT, eT, mgs[(half, j // 4)][:, j % 4, :])
                return eT

            def emit_vstrip(m):
                # v projection for one 128-token strip (token-major + ones);
                # psum rides the otherwise-idle projps bank
                grp, m8 = m // 8, m % 8
                pv = psA.tile([128, DH], F32, name="pv", tag="projps",
                              bufs=2)
                for kc in range(NK + 1):
                    if kc < NK:
                        lhsT = xvs[grp][:, kc, m8 * 128:(m8 + 1) * 128]
                    else:
                        lhsT = ones[0:1, 0:128]
                    nc.tensor.matmul(
                        out=pv[:, :],
                        lhsT=lhsT,
                        rhs=wvS[0:(128 if kc < NK else 1), kc, :],
                        start=(kc == 0), stop=(kc == NK))
                nc.vector.tensor_copy(
                    out=vta[:, m, :, 0:DK],
                    in_=pv[:, :].rearrange("p (h d) -> p h d", h=HL))

            def emit_scores_range(i, j0, j1, hooks=None):
                half, h = units[i]
                eTs = eTs_of.setdefault(i, [])
                for j in range(j0, j1):
                    eTs.append(emit_smem(h, half, j, (h % 2) * DK, h // 2))
                    if hooks and j in hooks:
                        hooks[j]()

            def emit_scores_unit(i, hooks=None):
                emit_scores_range(i, 0, NJ, hooks)

            def make_accs():
                # two acc tiles (1 PSUM bank each) hold the 8 q-chunk
                # accumulators; a zero-matmul clears each bank so the
                # per-chunk accumulation can run with start=False
                accs = []
                for a in range(2):
                    acc = psB.tile([128, 4, 128], F32, name="acc",
                                   tag="small", bufs=2)
                    nc.tensor.matmul(
                        out=acc[:, :, :], lhsT=zlhs[0:1, 0:128],
                        rhs=ones[0:1, 0:SC], start=True, stop=True)
                    accs.append(acc)
                return accs

            def pv_mm(i, h, accs, j, qc):
                nc.tensor.matmul(
                    out=accs[qc // 4][:, qc % 4, 0:DK + 1],
                    lhsT=eTs_of[i][j][:, qc * 128:(qc + 1) * 128],
                    rhs=vta[:, j, h, :],
                    start=False, stop=(j == NJ - 1),
                    skip_group_check=True)

            def emit_pv_strips(i, accs, jr, with_v=False):
                half, h = units[i]
                for j in jr:
                    if with_v:
                        # v-strip j feeds pv strip j directly: an xv DMA
                        # stall here never blocks a score stream
                        emit_vstrip(j)
                    for qc in range(NQC):
                        pv_mm(i, h, accs, j, qc)

            xps_of = {}

            def emit_tp(half, h, xps, qc, tail=False):
                # transpose one head-pair x tile chunk to feature-major xfin
                pair, off = h // 2, half * HS
                tp = psB.tile([128, 8, 128], F16, name="tp",
                              tag="small", bufs=2)
                nc.tensor.transpose(tp[:, 0, :], xps[qc][:, :, :], identS)
                nc.vector.tensor_copy(
                    out=xfin[:, pair, off + qc * 128:off + (qc + 1) * 128],
                    in_=tp[:, 0, :])

            def emit_norm_unit(i, accs, fuse_tail=False):
                half, h = units[i]
                if h % 2 == 0:
                    xps = [pb.tile([128, 2, DK], F16, name="xp",
                                   tag="xp", bufs=16) for _ in range(NQC)]
                    xps_of[h // 2] = xps
                else:
                    xps = xps_of[h // 2]
                for qc in range(NQC):
                    acc = accs[qc // 4]
                    rec = pb.tile([128, 1], F32, name="rec", tag="rec",
                                  bufs=8)
                    with nc.allow_low_precision(
                            reason="softmax denom reciprocal"):
                        nc.vector.reciprocal(rec, acc[:, qc % 4, DK:DK + 1])
                        nc.vector.tensor_scalar_mul(
                            xps[qc][:, h % 2, :], acc[:, qc % 4, 0:DK], rec)
                    if fuse_tail:
                        # final unit: stream norm -> transpose -> out-proj
                        # per q-chunk to shorten the kernel tail
                        emit_tp(half, h, xps, qc, tail=True)
                        out_proj(NQC + qc, ("act", "dve"), tail=True,
                                 alt=qc % 2)
                if h % 2 == 1 and not fuse_tail:
                    for qc in range(NQC):
                        emit_tp(half, h, xps, qc)

            def out_proj(m, engs, tail=False, alt=0):
                # phase C: one 128-token output strip.  During phase B the
                # psum rides half-bank tiles on the "small" ring (keeps the
                # sT double-buffer undisturbed); at the tail the score ring
                # is idle, so a full-width tile + one eviction is cheaper.
                ost = pb.tile([128, D], BF16, name="ost", tag="ost", bufs=2)
                if tail:
                    po = psB.tile([128, D], F32, name="pot",
                                  tag="big", bufs=2)
                    for kc in range(2):
                        for n2 in range(2):
                            nc.tensor.matmul(
                                out=po[:, n2 * SC:(n2 + 1) * SC],
                                lhsT=xfin[:, kc, m * 128:(m + 1) * 128],
                                rhs=woS[:, kc, n2 * SC:(n2 + 1) * SC],
                                start=(kc == 0), stop=(kc == 1))
                    if alt:
                        nc.scalar.activation(ost, po, AF.Copy)
                    else:
                        nc.vector.tensor_copy(out=ost, in_=po)
                else:
                    for n2 in range(2):
                        po = psB.tile([128, SC], F32, name="po2",
                                      tag="small", bufs=2)
                        for kc in range(2):
                            nc.tensor.matmul(
                                out=po,
                                lhsT=xfin[:, kc, m * 128:(m + 1) * 128],
                                rhs=woS[:, kc, n2 * SC:(n2 + 1) * SC],
                                start=(kc == 0), stop=(kc == 1))
                        oslice = ost[:, n2 * SC:(n2 + 1) * SC]
                        if engs[n2] == "act":
                            nc.scalar.activation(oslice, po, AF.Copy)
                        else:
                            nc.vector.tensor_copy(out=oslice, in_=po)
                nc.sync.dma_start(out=out[m * 128:(m + 1) * 128, :],
                                  in_=ost)

            # --- software-pipelined phase B schedule ---
            # Unit u's scores are emitted between the two halves of unit
            # u-1's PV so the eT ring slots for u's first exps free up
            # before u's score stream occupies the PE queue.
            # scores run two units ahead of PV so neither the v-projection
            # block nor a PV stream ever delays the exp-feeding score
            # stream in the in-order PE queue
            emit_scores_unit(0, hooks={
                3: lambda: proj_half("k", xk0, wkS, kT, 0,
                                     ("dve", "dve"), ms=(1,)),
                5: lambda: proj_half("k", xk1, wkS, kT, 1,
                                     ("dve", "dve"), ms=(0,)),
                12: lambda: proj_half("k", xk1, wkS, kT, 1,
                                      ("dve", "dve"), ms=(1,))})
            emit_scores_unit(1)
            emit_scores_unit(2)
            accs0 = make_accs()
            emit_pv_strips(0, accs0, range(NJ), with_v=True)
            emit_norm_unit(0, accs0)
            pend = [1, 2]
            op_m = 0
            for i in range(3, len(units)):
                pi = pend.pop(0)
                paccs = make_accs()
                emit_pv_strips(pi, paccs, range(0, NJ // 2))
                emit_scores_unit(i)
                emit_pv_strips(pi, paccs, range(NJ // 2, NJ))
                emit_norm_unit(pi, paccs)
                if pi == 1:
                    # half-1 q projection, needed from unit 4 on
                    proj_half("q", xq1, wqS, qT, 1, ("dve", "dve"))
                if pi >= 3:
                    # out-proj strips interleave once half 0 is complete
                    # (unit 3 closes half 0's xfin)
                    for _ in range(2):
                        out_proj(op_m, ("dve", "dve"))
                        op_m += 1
                pend.append(i)
            pi = pend.pop(0)
            paccs = make_accs()
            emit_pv_strips(pi, paccs, range(NJ))
            emit_norm_unit(pi, paccs)
            for _ in range(2):
                out_proj(op_m, ("dve", "dve"))
                op_m += 1
            # final unit: qc-outer PV fused with norm -> transpose ->
            # out-proj per q-chunk, so the tail pipeline starts with the
            # first q-chunk instead of after the whole PV block
            li = pend.pop(0)
            laccs = make_accs()
            lhalf, lh = units[li]
            lxps = xps_of[lh // 2]
            # alternate acc tiles (qc 0,4,1,5,...) so each chunk's PV
            # matmuls overlap the other tile's normalize chain instead of
            # serializing on whole-tile read/write ordering
            for qc in [x for p in zip(range(4), range(4, 8)) for x in p]:
                for j in range(NJ):
                    pv_mm(li, lh, laccs, j, qc)
                acc = laccs[qc // 4]
                rec = pb.tile([128, 1], F32, name="rec", tag="rec", bufs=8)
                with nc.allow_low_precision(
                        reason="softmax denom reciprocal"):
                    nc.vector.reciprocal(rec, acc[:, qc % 4, DK:DK + 1])
                    nc.vector.tensor_scalar_mul(
                        lxps[qc][:, lh % 2, :], acc[:, qc % 4, 0:DK], rec)
                emit_tp(lhalf, lh, lxps, qc, tail=True)
                out_proj(NQC + qc, ("act", "dve"), tail=True, alt=qc % 2)
            pb.release()
            psB.release()
            psA.release()
            pa.release()
    nc.finalize()
    return nc


def _get_nc():
    if "nc" not in _CACHE:
        _CACHE["nc"] = _build()
    return _CACHE["nc"]


def _prep_in_maps(query, key_, value, mask, Wq, bq, Wk, bk, Wv, bv, Wo, bo):
    query = np.asarray(query, np.float32)
    key_ = np.asarray(key_, np.float32)
    value = np.asarray(value, np.float32)
    mask = np.asarray(mask)
    Wq, bq = np.asarray(Wq, np.float32), np.asarray(bq, np.float32)
    Wk, bk = np.asarray(Wk, np.float32), np.asarray(bk, np.float32)
    Wv, bv = np.asarray(Wv, np.float32), np.asarray(bv, np.float32)
    Wo = np.asarray(Wo, np.float32)

    xT = {}
    for b in range(B):
        xT[("q", b)] = np.ascontiguousarray(query[b].T).astype(np.float16)
        xT[("k", b)] = np.ascontiguousarray(key_[b].T).astype(np.float16)
        xT[("v", b)] = np.ascontiguousarray(value[b].T).astype(np.float16)
        xT[("m", b)] = np.ascontiguousarray(mask[b].T).astype(
            ml_dtypes.bfloat16)
    identity = np.eye(128, dtype=np.float16)
    wg = {}
    for g in range(GROUPS):
        c0, c1 = g * DH, (g + 1) * DH
        wg[("q", g)] = np.ascontiguousarray(Wq[:, c0:c1]).astype(np.float16)
        wg[("k", g)] = np.ascontiguousarray(Wk[:, c0:c1]).astype(np.float16)
        wg[("v", g)] = np.concatenate(
            [Wv[:, c0:c1], bv[None, c0:c1]], axis=0).astype(np.float16)
        wg[("o", g)] = np.ascontiguousarray(Wo[c0:c1, :]).astype(np.float16)
        wg[("bqk", g)] = np.stack(
            [bq[c0:c0 + 128], bq[c0 + 128:c1],
             bk[c0:c0 + 128], bk[c0 + 128:c1]], axis=1).astype(np.float32)

    in_maps = []
    for c in range(NCORES):
        b, g = c // GROUPS, c % GROUPS
        in_maps.append({
            "xqT": xT[("q", b)], "xkT": xT[("k", b)], "xvT": xT[("v", b)],
            "mT": xT[("m", b)],
            "wq": wg[("q", g)], "wk": wg[("k", g)], "wv": wg[("v", g)],
            "wo": wg[("o", g)], "bqk": wg[("bqk", g)],
            "ident": identity,
        })
    return in_maps


def _gather(results, bo):
    bo = np.asarray(bo, np.float32)
    outs = []
    for b in range(B):
        acc = results[b * GROUPS]["out"].astype(np.float32).copy()
        for g in range(1, GROUPS):
            acc += results[b * GROUPS + g]["out"]
        outs.append(acc + bo[None, :])
    return np.stack(outs, axis=0)


def run(trace=False, **inputs):
    in_maps = _prep_in_maps(**inputs)
    nc = _get_nc()
    res = run_bass_kernel_spmd(nc, in_maps, core_ids=list(range(NCORES)),
                               trace=trace)
    out = _gather(res.results, inputs["bo"])
    return out, res


def kernel(**inputs) -> np.ndarray:
    out, _ = run(trace=False, **inputs)
    return out
